# revision 4
# baseline (speedup 1.0000x reference)
import os
import sys

import numpy as np

for p in ("/opt/trn_rl_repo",):
    if p not in sys.path:
        sys.path.insert(0, p)

import concourse.bass as bass  # noqa: E402
import concourse.tile as tile  # noqa: E402
from concourse import bacc, mybir  # noqa: E402
from concourse.bass_utils import run_bass_kernel_spmd  # noqa: E402

B, N, D = 128, 512, 512
NCORES = 8
BPC = B // NCORES  # 16 batch items per core
F32 = mybir.dt.float32
F32R = mybir.dt.float32r

LAST_RESULTS = None


def _hadamard(n: int) -> np.ndarray:
    H = np.array([[1.0]], dtype=np.float32)
    base = np.array([[1.0, 1.0], [1.0, -1.0]], dtype=np.float32)
    while H.shape[0] < n:
        H = np.kron(H, base)
    return H


def _build():
    nc = bacc.Bacc("TRN2", target_bir_lowering=False, debug=False)
    # x/y as [BPC, 128, 2048]: same bytes as [BPC, 512, 512], partition p
    # holds rows 4p..4p+3 (column block k of 512 = row 4p+k).
    x_d = nc.dram_tensor("x", [BPC, 128, 4 * D], F32R, kind="ExternalInput").ap()
    # h1[p, k*512 + l*128 + q] = H[4p+k, 4q+l]
    h1_d = nc.dram_tensor("h1", [128, 4 * N], F32R, kind="ExternalInput").ap()
    # hs[c, dt*512 + e] = H[dt*128+c, e] / 512
    hs_d = nc.dram_tensor("hs", [128, 4 * N], F32R, kind="ExternalInput").ap()
    y_d = nc.dram_tensor("y", [BPC, 128, 4 * D], F32, kind="ExternalOutput").ap()

    with tile.TileContext(nc) as tc:
        with (
            tc.tile_pool(name="const", bufs=1) as const_pool,
            tc.tile_pool(name="xp", bufs=3) as x_pool,
            tc.tile_pool(name="tp", bufs=2) as t_pool,
            tc.tile_pool(name="yp", bufs=3) as y_pool,
            tc.tile_pool(name="ps", bufs=8, space="PSUM") as psum_pool,
        ):
            h1_sb = const_pool.tile([128, 4 * N], F32R, tag="h1")
            nc.sync.dma_start(h1_sb[:], h1_d[:])
            hs_sb = const_pool.tile([128, 4 * N], F32R, tag="hs")
            nc.sync.dma_start(hs_sb[:], hs_d[:])

            for b in range(BPC):
                xt = x_pool.tile([128, 4 * D], F32R)
                nc.sync.dma_start(xt[:], x_d[b])

                # Pass 1: tT[d, n] = sum_m x[m, d] * H[m, n], d-major in
                # partitions.  tt[c, dt*512 + l*128 + q] = tT[dt*128+c, 4q+l]
                tt = t_pool.tile([128, 4 * N], F32R)
                for dt_ in range(4):
                    ps = psum_pool.tile([128, N], F32)
                    for k in range(4):
                        nc.tensor.matmul(
                            ps[:],
                            xt[:, k * D + dt_ * 128 : k * D + dt_ * 128 + 128],
                            h1_sb[:, k * N : (k + 1) * N],
                            start=(k == 0),
                            stop=(k == 3),
                        )
                    dst = tt[:, dt_ * N : (dt_ + 1) * N]
                    if dt_ % 2 == 0:
                        nc.vector.tensor_copy(dst, ps[:])
                    else:
                        nc.scalar.copy(dst, ps[:])

                # Pass 2: y[4p+k2, e] = sum_d tT[d, 4p+k2] * (H/512)[d, e]
                yt = y_pool.tile([128, 4 * D], F32)
                for k2 in range(4):
                    ps = psum_pool.tile([128, D], F32)
                    for dt_ in range(4):
                        nc.tensor.matmul(
                            ps[:],
                            tt[:, dt_ * N + k2 * 128 : dt_ * N + k2 * 128 + 128],
                            hs_sb[:, dt_ * D : (dt_ + 1) * D],
                            start=(dt_ == 0),
                            stop=(dt_ == 3),
                        )
                    dst = yt[:, k2 * D : (k2 + 1) * D]
                    if k2 % 2 == 0:
                        nc.scalar.copy(dst, ps[:])
                    else:
                        nc.vector.tensor_copy(dst, ps[:])
                nc.sync.dma_start(y_d[b], yt[:])

    nc.compile()
    return nc


_NC = None


def kernel(x: np.ndarray) -> np.ndarray:
    global _NC, LAST_RESULTS
    if _NC is None:
        _NC = _build()
    x = np.ascontiguousarray(np.asarray(x), dtype=np.float32).reshape(
        NCORES, BPC, 128, 4 * D
    )
    H = _hadamard(N)
    h1 = np.ascontiguousarray(
        H.reshape(128, 4, 128, 4).transpose(0, 1, 3, 2).reshape(128, 4 * N)
    )
    hs = np.ascontiguousarray(
        H.reshape(4, 128, N).transpose(1, 0, 2).reshape(128, 4 * N)
        / np.float32(512.0)
    )
    in_maps = [
        {"x": x[i], "h1": h1, "hs": hs} for i in range(NCORES)
    ]
    trace = os.environ.get("KERNEL_TRACE", "") == "1"
    res = run_bass_kernel_spmd(
        _NC, in_maps, list(range(NCORES)), trace=trace
    )
    LAST_RESULTS = res
    out = np.stack([r["y"] for r in res.results], axis=0)
    return out.reshape(B, N, D).astype(np.float32)
